# revision 1
# baseline (speedup 1.0000x reference)
"""KeyedLSTM Trainium2 kernel.

Strategy: tensor-parallel split of the 4H gate dimension across 8 cores.
Each core holds W_ih/W_hh column slices for its 256 h-rows (of each of the
i/f/o/g gate blocks) resident in SBUF, computes its slice of the gates /
c / h each step, and an AllGather of h (2048x32 fp32) runs every step so
every core has the full h for the next step's recurrent matmul.

The x @ W_ih.T + bias part has no recurrence, so it is precomputed for all
B*S tokens as an efficient bulk matmul (float32r, N=512) into DRAM and
streamed back 128KB/step during the recurrence.
"""

import os
import sys

import numpy as np

for _p in (
    "/root/.axon_site",
    "/root/.axon_site/_ro/trn_rl_repo",
    "/root/.axon_site/_ro/pypackages",
    "/opt/trn_rl_repo",
):
    if os.path.isdir(_p) and _p not in sys.path:
        sys.path.append(_p)

import concourse.bacc as bacc
import concourse.bass_utils as bass_utils
import concourse.mybir as mybir
import concourse.tile as tile

AF = mybir.ActivationFunctionType
ALU = mybir.AluOpType
DT = mybir.dt

B, S, I, H = 32, 256, 1024, 2048
KB, KL = 4, 16
NCORES = 8
HLOC = H // NCORES  # 256 h rows per core
MT = 8  # m-tiles of 128 gate rows per core
KT_I = I // 128  # 8
KT_H = H // 128  # 16

_GOFF = {"i": 0, "f": H, "g": 2 * H, "o": 3 * H}
_ORDER = ("i", "f", "o", "g")  # m-tile order within each 128-row sub-block


def _rows_for_core(j):
    rows = []
    for p in range(2):
        base = j * HLOC + p * 128
        for g in _ORDER:
            o = _GOFF[g] + base
            rows.extend(range(o, o + 128))
    return np.asarray(rows, dtype=np.int64)


def _build_program(s_steps):
    """One SPMD program, identical on all cores; weights differ per-core."""
    ttok = s_steps * B

    nc = bacc.Bacc(
        "TRN2",
        target_bir_lowering=False,
        debug=False,
        enable_asserts=True,
        num_devices=NCORES,
    )

    xt = nc.dram_tensor("xt", [I, ttok], DT.float32r, kind="ExternalInput").ap()
    kt = nc.dram_tensor("kt", [I, KL * KB], DT.float32r, kind="ExternalInput").ap()
    wih = nc.dram_tensor("wih", [I, MT * 128], DT.float32r, kind="ExternalInput").ap()
    whh = nc.dram_tensor("whh", [H, MT * 128], DT.float32, kind="ExternalInput").ap()
    bias = nc.dram_tensor("bias", [MT * 128], DT.float32, kind="ExternalInput").ap()
    out = nc.dram_tensor(
        "out", [s_steps, 2 * 128, B], DT.float32, kind="ExternalOutput"
    ).ap()

    rg = [list(range(NCORES))]

    with tile.TileContext(nc) as tc:
        with (
            tc.tile_pool(name="const", bufs=1) as const_pool,
            tc.tile_pool(name="dram", bufs=1, space="DRAM") as dram_pool,
        ):
            # ---- resident tensors ----
            whh_sb = const_pool.tile([128, KT_H, MT, 128], DT.float32)
            nc.sync.dma_start(whh_sb[:], whh.rearrange("(k p) m -> p k m", p=128))
            bias_sb = const_pool.tile([128, MT], DT.float32)
            nc.sync.dma_start(bias_sb[:], bias.rearrange("(m p) -> p m", p=128))

            xw_dram = dram_pool.tile([MT * 128, ttok], DT.float32)

            h_sb = const_pool.tile([128, KT_H, B], DT.float32)
            c_sb = const_pool.tile([128, 2, B], DT.float32)
            hk_sb = const_pool.tile([128, KT_H, KB], DT.float32)
            ck_sb = const_pool.tile([128, 2, KB], DT.float32)
            mult_sb = const_pool.tile([128, KL, 2], DT.float32)
            xk_sb = const_pool.tile([128, MT, KL * KB], DT.float32)
            nc.vector.memset(h_sb[:], 0.0)
            nc.vector.memset(c_sb[:], 0.0)
            nc.vector.memset(hk_sb[:], 0.0)
            nc.vector.memset(ck_sb[:], 0.0)

            # ---- phase 1: XW / XK precompute (x @ W_ih.T + bias) ----
            with (
                tc.tile_pool(name="wih_pool", bufs=1) as wih_pool,
                tc.tile_pool(name="xin", bufs=2) as xin_pool,
                tc.tile_pool(name="xw_ps", bufs=4, space="PSUM") as xw_ps_pool,
                tc.tile_pool(name="xw_st", bufs=3) as xw_st_pool,
            ):
                wih_sb = wih_pool.tile([128, KT_I, MT, 128], DT.float32r)
                nc.sync.dma_start(wih_sb[:], wih.rearrange("(k p) m -> p k m", p=128))

                # key-seq x-part (tiny, kept in SBUF)
                k_sb = xin_pool.tile([128, KT_I, KL * KB], DT.float32r, tag="kin")
                nc.sync.dma_start(k_sb[:], kt.rearrange("(k p) t -> p k t", p=128))
                for m in range(MT):
                    ps = xw_ps_pool.tile([128, 512], DT.float32, tag="ps")
                    for k in range(KT_I):
                        nc.tensor.matmul(
                            ps[:, : KL * KB],
                            wih_sb[:, k, m, :],
                            k_sb[:, k, :],
                            start=(k == 0),
                            stop=(k == KT_I - 1),
                        )
                    nc.scalar.activation(
                        xk_sb[:, m, :],
                        ps[:, : KL * KB],
                        AF.Identity,
                        bias=bias_sb[:, m : m + 1],
                    )

                # main x-part, bulk over all tokens
                chunk = min(512, ttok)
                assert ttok % chunk == 0
                for n in range(ttok // chunk):
                    x_sb = xin_pool.tile([128, KT_I, chunk], DT.float32r, tag="xin")
                    nc.sync.dma_start(
                        x_sb[:],
                        xt.rearrange("(k p) t -> p k t", p=128)[
                            :, :, n * chunk : (n + 1) * chunk
                        ],
                    )
                    for m in range(MT):
                        ps = xw_ps_pool.tile([128, 512], DT.float32, tag="ps")
                        for k in range(KT_I):
                            nc.tensor.matmul(
                                ps[:, :chunk],
                                wih_sb[:, k, m, :],
                                x_sb[:, k, :],
                                start=(k == 0),
                                stop=(k == KT_I - 1),
                            )
                        xw_st = xw_st_pool.tile([128, chunk], DT.float32, tag="st")
                        nc.scalar.activation(
                            xw_st[:],
                            ps[:, :chunk],
                            AF.Identity,
                            bias=bias_sb[:, m : m + 1],
                        )
                        nc.sync.dma_start(
                            xw_dram[m * 128 : (m + 1) * 128, n * chunk : (n + 1) * chunk],
                            xw_st[:],
                        )

            # ---- phase 2: key recurrence (collect forget-gate means) ----
            with (
                tc.tile_pool(name="kps", bufs=2, space="PSUM") as kps_pool,
                tc.tile_pool(name="ktmp", bufs=2) as ktmp_pool,
                tc.tile_pool(name="kdram", bufs=2, space="DRAM") as kdram_pool,
            ):
                for t in range(KL):
                    ps = kps_pool.tile([128, MT, KB], DT.float32, tag="kps")
                    for m in range(MT):
                        for k in range(KT_H):
                            nc.tensor.matmul(
                                ps[:, m, :],
                                whh_sb[:, k, m, :],
                                hk_sb[:, k, :],
                                start=(k == 0),
                                stop=(k == KT_H - 1),
                            )
                    gpre = ktmp_pool.tile([128, MT, KB], DT.float32, tag="gpre")
                    nc.vector.tensor_add(
                        gpre[:], ps[:], xk_sb[:, :, t * KB : (t + 1) * KB]
                    )
                    gact = ktmp_pool.tile([128, MT, KB], DT.float32, tag="gact")
                    gpre_v = gpre.rearrange("p (s g) b -> p s g b", s=2)
                    gact_v = gact.rearrange("p (s g) b -> p s g b", s=2)
                    nc.scalar.activation(
                        gact_v[:, :, 0:3, :], gpre_v[:, :, 0:3, :], AF.Sigmoid
                    )
                    nc.scalar.activation(
                        gact_v[:, :, 3, :], gpre_v[:, :, 3, :], AF.Tanh
                    )
                    i_v = gact_v[:, :, 0, :]
                    f_v = gact_v[:, :, 1, :]
                    o_v = gact_v[:, :, 2, :]
                    g_v = gact_v[:, :, 3, :]
                    t1 = ktmp_pool.tile([128, 2, KB], DT.float32, tag="t1")
                    t2 = ktmp_pool.tile([128, 2, KB], DT.float32, tag="t2")
                    nc.vector.tensor_mul(t1[:], i_v, g_v)
                    nc.vector.tensor_mul(t2[:], f_v, ck_sb[:])
                    nc.vector.tensor_add(ck_sb[:], t1[:], t2[:])
                    fs = ktmp_pool.tile([128, 2], DT.float32, tag="fs")
                    nc.vector.tensor_reduce(fs[:], f_v, mybir.AxisListType.X, ALU.add)
                    nc.vector.tensor_scalar_mul(mult_sb[:, t, :], fs[:], 1.0 / KB)
                    th = ktmp_pool.tile([128, 2, KB], DT.float32, tag="th")
                    nc.scalar.activation(th[:], ck_sb[:], AF.Tanh)
                    hloc = ktmp_pool.tile([128, 2, KB], DT.float32, tag="hloc")
                    nc.vector.tensor_mul(hloc[:], o_v, th[:])
                    ag_in = kdram_pool.tile([2 * 128, KB], DT.float32, tag="agin")
                    nc.sync.dma_start(
                        ag_in.rearrange("(s p) b -> p s b", p=128), hloc[:]
                    )
                    ag_out = kdram_pool.tile(
                        [H, KB], DT.float32, tag="agout", addr_space="Shared"
                    )
                    nc.gpsimd.collective_compute(
                        "AllGather",
                        ALU.bypass,
                        replica_groups=rg,
                        ins=[ag_in.opt()],
                        outs=[ag_out.opt()],
                    )
                    nc.sync.dma_start(
                        hk_sb[:], ag_out.rearrange("(k p) b -> p k b", p=128)
                    )

            # ---- phase 3: main recurrence ----
            with (
                tc.tile_pool(name="mps", bufs=4, space="PSUM") as mps_pool,
                tc.tile_pool(name="mtmp", bufs=2) as mtmp_pool,
                tc.tile_pool(name="xwt", bufs=3) as xwt_pool,
                tc.tile_pool(name="mdram", bufs=2, space="DRAM") as mdram_pool,
            ):
                xw_re = xw_dram.rearrange("(m p) t -> p m t", p=128)
                for t in range(s_steps):
                    xw_t = xwt_pool.tile([128, MT, B], DT.float32, tag="xwt")
                    nc.sync.dma_start(xw_t[:], xw_re[:, :, t * B : (t + 1) * B])

                    ps = mps_pool.tile([128, MT, B], DT.float32, tag="mps")
                    for m in range(MT):
                        for k in range(KT_H):
                            nc.tensor.matmul(
                                ps[:, m, :],
                                whh_sb[:, k, m, :],
                                h_sb[:, k, :],
                                start=(k == 0),
                                stop=(k == KT_H - 1),
                            )
                    gpre = mtmp_pool.tile([128, MT, B], DT.float32, tag="gpre")
                    nc.vector.tensor_add(gpre[:], ps[:], xw_t[:])
                    gact = mtmp_pool.tile([128, MT, B], DT.float32, tag="gact")
                    gpre_v = gpre.rearrange("p (s g) b -> p s g b", s=2)
                    gact_v = gact.rearrange("p (s g) b -> p s g b", s=2)
                    nc.scalar.activation(
                        gact_v[:, :, 0:3, :], gpre_v[:, :, 0:3, :], AF.Sigmoid
                    )
                    nc.scalar.activation(
                        gact_v[:, :, 3, :], gpre_v[:, :, 3, :], AF.Tanh
                    )
                    i_v = gact_v[:, :, 0, :]
                    f_v = gact_v[:, :, 1, :]
                    o_v = gact_v[:, :, 2, :]
                    g_v = gact_v[:, :, 3, :]
                    t1 = mtmp_pool.tile([128, 2, B], DT.float32, tag="t1")
                    t2 = mtmp_pool.tile([128, 2, B], DT.float32, tag="t2")
                    nc.vector.tensor_mul(t1[:], i_v, g_v)
                    nc.vector.tensor_mul(t2[:], f_v, c_sb[:])
                    nc.vector.tensor_add(c_sb[:], t1[:], t2[:])
                    th = mtmp_pool.tile([128, 2, B], DT.float32, tag="th")
                    nc.scalar.activation(th[:], c_sb[:], AF.Tanh)
                    out_h = mtmp_pool.tile([128, 2, B], DT.float32, tag="outh")
                    nc.vector.tensor_mul(out_h[:], o_v, th[:])
                    nc.sync.dma_start(
                        out[t].rearrange("(s p) b -> p s b", p=128), out_h[:]
                    )
                    if t < KL:
                        h_nx = mtmp_pool.tile([128, 2, B], DT.float32, tag="hnx")
                        for p in range(2):
                            nc.vector.tensor_scalar_mul(
                                h_nx[:, p, :], out_h[:, p, :], mult_sb[:, t, p : p + 1]
                            )
                            nc.vector.tensor_scalar_mul(
                                c_sb[:, p, :], c_sb[:, p, :], mult_sb[:, t, p : p + 1]
                            )
                        h_send = h_nx
                    else:
                        h_send = out_h
                    ag_in = mdram_pool.tile([2 * 128, B], DT.float32, tag="agin")
                    nc.sync.dma_start(
                        ag_in.rearrange("(s p) b -> p s b", p=128), h_send[:]
                    )
                    ag_out = mdram_pool.tile(
                        [H, B], DT.float32, tag="agout", addr_space="Shared"
                    )
                    nc.gpsimd.collective_compute(
                        "AllGather",
                        ALU.bypass,
                        replica_groups=rg,
                        ins=[ag_in.opt()],
                        outs=[ag_out.opt()],
                    )
                    nc.sync.dma_start(
                        h_sb[:], ag_out.rearrange("(k p) b -> p k b", p=128)
                    )

    nc.compile()
    return nc


def _prepare_inputs(x, key_seq, weight_ih, weight_hh, bias_ih, bias_hh, s_steps):
    x = np.ascontiguousarray(np.asarray(x, dtype=np.float32)[:, :s_steps, :])
    key_seq = np.asarray(key_seq, dtype=np.float32)
    weight_ih = np.asarray(weight_ih, dtype=np.float32)
    weight_hh = np.asarray(weight_hh, dtype=np.float32)
    b = (np.asarray(bias_ih, dtype=np.float32) + np.asarray(bias_hh, dtype=np.float32))

    # tokens ordered (s, b): column s*B + b
    xt = np.ascontiguousarray(x.transpose(2, 1, 0).reshape(I, s_steps * B))
    kt = np.ascontiguousarray(key_seq.transpose(2, 1, 0).reshape(I, KL * KB))

    in_maps = []
    for j in range(NCORES):
        rows = _rows_for_core(j)
        in_maps.append(
            {
                "xt": xt,
                "kt": kt,
                "wih": np.ascontiguousarray(weight_ih[rows].T),
                "whh": np.ascontiguousarray(weight_hh[rows].T),
                "bias": np.ascontiguousarray(b[rows]),
            }
        )
    return in_maps


_NC_CACHE = {}


def _run(x, key_seq, weight_ih, weight_hh, bias_ih, bias_hh, s_steps, trace=False):
    if s_steps not in _NC_CACHE:
        _NC_CACHE[s_steps] = _build_program(s_steps)
    nc = _NC_CACHE[s_steps]
    in_maps = _prepare_inputs(
        x, key_seq, weight_ih, weight_hh, bias_ih, bias_hh, s_steps
    )
    res = bass_utils.run_bass_kernel_spmd(
        nc, in_maps, core_ids=list(range(NCORES)), trace=trace
    )
    # out_j: [s, r, b] with global h row = j*HLOC + r
    pieces = [res.results[j]["out"].transpose(0, 2, 1) for j in range(NCORES)]
    full = np.concatenate(pieces, axis=2)  # (s_steps, B, H)
    return full, res


def kernel(x, key_seq, weight_ih, weight_hh, bias_ih, bias_hh):
    s_steps = int(os.environ.get("KEYED_LSTM_STEPS", S))
    trace = os.environ.get("KEYED_LSTM_TRACE", "0") == "1"
    full, _res = _run(
        x, key_seq, weight_ih, weight_hh, bias_ih, bias_hh, s_steps, trace=trace
    )
    return full


# revision 2
# speedup vs baseline: 3.0217x; 3.0217x over previous
"""KeyedLSTM Trainium2 kernel.

Strategy: tensor-parallel split of the 4H gate dimension across 8 cores.
Each core holds W_ih/W_hh column slices for its 256 h-rows (of each of the
g/i/f/o gate blocks) resident in SBUF, computes its slice of the gates /
c / h each step, and an AllGather of h (2048x32) runs every step so every
core has the full h for the next step's recurrent matmul.

Precision: the recurrent matmul runs in fp16 (weights + h) with fp32 PSUM
accumulation — fp16 keeps a 10-bit mantissa (better than bf16 for these
small-magnitude operands) and gets single-pass matmuls + fast weight load
(fp32 matmuls lower to 2 LDWEIGHTS+MATMUL passes and run ~4x slower).
c, the gates, and the kernel output stay fp32.

The x @ W_ih.T + bias part has no recurrence, so it is precomputed for all
B*S tokens as an efficient bulk matmul (float32r, N=512) into DRAM and
streamed back 128KB/step during the recurrence.

Per-step layout: each core's 1024 gate rows are ordered [g,i,f,o] per
128-row h sub-block, split into two PSUM banks (sub0 = m0..3, sub1 =
m4..7) so the activation/cell-update chain for sub0 overlaps the PE
matmuls of sub1, and the o-gate is last so the c-chain runs under the
o matmuls.
"""

import os
import sys

import numpy as np

for _p in (
    "/root/.axon_site",
    "/root/.axon_site/_ro/trn_rl_repo",
    "/root/.axon_site/_ro/pypackages",
    "/opt/trn_rl_repo",
):
    if os.path.isdir(_p) and _p not in sys.path:
        sys.path.append(_p)

import concourse.bacc as bacc
import concourse.bass_utils as bass_utils
import concourse.mybir as mybir
import concourse.tile as tile

AF = mybir.ActivationFunctionType
ALU = mybir.AluOpType
DT = mybir.dt

B, S, I, H = 32, 256, 1024, 2048
KB, KL = 4, 16
NCORES = 8
HLOC = H // NCORES  # 256 h rows per core
MT = 8  # m-tiles of 128 gate rows per core
KT_I = I // 128  # 8
KT_H = H // 128  # 16

_GOFF = {"i": 0, "f": H, "g": 2 * H, "o": 3 * H}
# m-tile order within each 128-row sub-block: tanh gate first (index 0) so
# sigmoid gates are a contiguous [1:4] slice, o last so the cell-update
# chain overlaps the o matmuls.
_ORDER = ("g", "i", "f", "o")


def _rows_for_core(j):
    rows = []
    for p in range(2):
        base = j * HLOC + p * 128
        for g in _ORDER:
            o = _GOFF[g] + base
            rows.extend(range(o, o + 128))
    return np.asarray(rows, dtype=np.int64)


def _build_program(s_steps):
    """One SPMD program, identical on all cores; weights differ per-core."""
    ttok = s_steps * B

    nc = bacc.Bacc(
        "TRN2",
        target_bir_lowering=False,
        debug=False,
        enable_asserts=True,
        num_devices=NCORES,
    )

    xt = nc.dram_tensor("xt", [I, ttok], DT.float32r, kind="ExternalInput").ap()
    kt = nc.dram_tensor("kt", [I, KL * KB], DT.float32r, kind="ExternalInput").ap()
    wih = nc.dram_tensor("wih", [I, MT * 128], DT.float32r, kind="ExternalInput").ap()
    whh = nc.dram_tensor("whh", [H, MT * 128], DT.float16, kind="ExternalInput").ap()
    bias = nc.dram_tensor("bias", [MT * 128], DT.float32, kind="ExternalInput").ap()
    out = nc.dram_tensor(
        "out", [s_steps, 2 * 128, B], DT.float32, kind="ExternalOutput"
    ).ap()

    rg = [list(range(NCORES))]

    with tile.TileContext(nc) as tc:
        with (
            tc.tile_pool(name="const", bufs=1) as const_pool,
            tc.tile_pool(name="dram", bufs=1, space="DRAM") as dram_pool,
        ):
            # ---- resident tensors ----
            whh_sb = const_pool.tile([128, KT_H, MT, 128], DT.float16)
            whh_re = whh.rearrange("(k p) m -> p k m", p=128)
            for g in range(4):
                nc.sync.dma_start(
                    whh_sb[:, 4 * g : 4 * (g + 1), :, :],
                    whh_re[:, 4 * g : 4 * (g + 1), :],
                )
            bias_sb = const_pool.tile([128, MT], DT.float32)
            nc.sync.dma_start(bias_sb[:], bias.rearrange("(m p) -> p m", p=128))

            xw_dram = dram_pool.tile([MT * 128, ttok], DT.float32)

            h_sb = const_pool.tile([128, KT_H, B], DT.float16)
            c_sb = const_pool.tile([128, 2, B], DT.float32)
            hk_sb = const_pool.tile([128, KT_H, KB], DT.float16)
            ck_sb = const_pool.tile([128, 2, KB], DT.float32)
            mult_sb = const_pool.tile([128, KL, 2], DT.float32)
            xk_sb = const_pool.tile([128, MT, KL * KB], DT.float32)
            nc.vector.memset(h_sb[:], 0.0)
            nc.vector.memset(c_sb[:], 0.0)
            nc.vector.memset(hk_sb[:], 0.0)
            nc.vector.memset(ck_sb[:], 0.0)

            # ---- phase 1: XW / XK precompute (x @ W_ih.T + bias) ----
            with (
                tc.tile_pool(name="wih_pool", bufs=1) as wih_pool,
                tc.tile_pool(name="xin", bufs=2) as xin_pool,
                tc.tile_pool(name="xw_ps", bufs=4, space="PSUM") as xw_ps_pool,
                tc.tile_pool(name="xw_st", bufs=3) as xw_st_pool,
            ):
                wih_sb = wih_pool.tile([128, KT_I, MT, 128], DT.float32r)
                wih_re = wih.rearrange("(k p) m -> p k m", p=128)
                for g in range(4):
                    nc.sync.dma_start(
                        wih_sb[:, 2 * g : 2 * (g + 1), :, :],
                        wih_re[:, 2 * g : 2 * (g + 1), :],
                    )

                # key-seq x-part (tiny, kept in SBUF)
                k_sb = xin_pool.tile([128, KT_I, KL * KB], DT.float32r, tag="kin")
                nc.sync.dma_start(k_sb[:], kt.rearrange("(k p) t -> p k t", p=128))
                for m in range(MT):
                    ps = xw_ps_pool.tile([128, 512], DT.float32, tag="ps")
                    for k in range(KT_I):
                        nc.tensor.matmul(
                            ps[:, : KL * KB],
                            wih_sb[:, k, m, :],
                            k_sb[:, k, :],
                            start=(k == 0),
                            stop=(k == KT_I - 1),
                        )
                    nc.scalar.activation(
                        xk_sb[:, m, :],
                        ps[:, : KL * KB],
                        AF.Identity,
                        bias=bias_sb[:, m : m + 1],
                    )

                # main x-part, bulk over all tokens
                chunk = min(512, ttok)
                assert ttok % chunk == 0
                for n in range(ttok // chunk):
                    x_sb = xin_pool.tile([128, KT_I, chunk], DT.float32r, tag="xin")
                    nc.sync.dma_start(
                        x_sb[:],
                        xt.rearrange("(k p) t -> p k t", p=128)[
                            :, :, n * chunk : (n + 1) * chunk
                        ],
                    )
                    for m in range(MT):
                        ps = xw_ps_pool.tile([128, 512], DT.float32, tag="ps")
                        for k in range(KT_I):
                            nc.tensor.matmul(
                                ps[:, :chunk],
                                wih_sb[:, k, m, :],
                                x_sb[:, k, :],
                                start=(k == 0),
                                stop=(k == KT_I - 1),
                            )
                        xw_st = xw_st_pool.tile([128, chunk], DT.float32, tag="st")
                        nc.scalar.activation(
                            xw_st[:],
                            ps[:, :chunk],
                            AF.Identity,
                            bias=bias_sb[:, m : m + 1],
                        )
                        nc.sync.dma_start(
                            xw_dram[m * 128 : (m + 1) * 128, n * chunk : (n + 1) * chunk],
                            xw_st[:],
                        )

            # ---- phase 2: key recurrence (collect forget-gate means) ----
            with (
                tc.tile_pool(name="kps", bufs=2, space="PSUM") as kps_pool,
                tc.tile_pool(name="ktmp", bufs=2) as ktmp_pool,
                tc.tile_pool(name="kdram", bufs=2, space="DRAM") as kdram_pool,
            ):
                for t in range(KL):
                    ps = kps_pool.tile([128, MT, KB], DT.float32, tag="kps")
                    for m in range(MT):
                        for k in range(KT_H):
                            nc.tensor.matmul(
                                ps[:, m, :],
                                whh_sb[:, k, m, :],
                                hk_sb[:, k, :],
                                start=(k == 0),
                                stop=(k == KT_H - 1),
                            )
                    gpre = ktmp_pool.tile([128, MT, KB], DT.float32, tag="gpre")
                    nc.vector.tensor_add(
                        gpre[:], ps[:], xk_sb[:, :, t * KB : (t + 1) * KB]
                    )
                    gact = ktmp_pool.tile([128, MT, KB], DT.float32, tag="gact")
                    gpre_v = gpre.rearrange("p (s g) b -> p s g b", s=2)
                    gact_v = gact.rearrange("p (s g) b -> p s g b", s=2)
                    nc.scalar.activation(
                        gact_v[:, :, 1:4, :], gpre_v[:, :, 1:4, :], AF.Sigmoid
                    )
                    nc.scalar.activation(
                        gact_v[:, :, 0, :], gpre_v[:, :, 0, :], AF.Tanh
                    )
                    g_v = gact_v[:, :, 0, :]
                    i_v = gact_v[:, :, 1, :]
                    f_v = gact_v[:, :, 2, :]
                    o_v = gact_v[:, :, 3, :]
                    t1 = ktmp_pool.tile([128, 2, KB], DT.float32, tag="t1")
                    t2 = ktmp_pool.tile([128, 2, KB], DT.float32, tag="t2")
                    nc.vector.tensor_mul(t1[:], i_v, g_v)
                    nc.vector.tensor_mul(t2[:], f_v, ck_sb[:])
                    nc.vector.tensor_add(ck_sb[:], t1[:], t2[:])
                    fs = ktmp_pool.tile([128, 2], DT.float32, tag="fs")
                    nc.vector.tensor_reduce(fs[:], f_v, mybir.AxisListType.X, ALU.add)
                    nc.vector.tensor_scalar_mul(mult_sb[:, t, :], fs[:], 1.0 / KB)
                    th = ktmp_pool.tile([128, 2, KB], DT.float32, tag="th")
                    nc.scalar.activation(th[:], ck_sb[:], AF.Tanh)
                    hloc = ktmp_pool.tile([128, 2, KB], DT.float16, tag="hloc")
                    nc.vector.tensor_mul(hloc[:], o_v, th[:])
                    ag_in = kdram_pool.tile([2 * 128, KB], DT.float16, tag="agin")
                    nc.sync.dma_start(
                        ag_in.rearrange("(s p) b -> p s b", p=128), hloc[:]
                    )
                    ag_out = kdram_pool.tile(
                        [H, KB], DT.float16, tag="agout", addr_space="Shared"
                    )
                    nc.gpsimd.collective_compute(
                        "AllGather",
                        ALU.bypass,
                        replica_groups=rg,
                        ins=[ag_in.opt()],
                        outs=[ag_out.opt()],
                    )
                    nc.sync.dma_start(
                        hk_sb[:], ag_out.rearrange("(k p) b -> p k b", p=128)
                    )

            # ---- phase 3: main recurrence ----
            with (
                tc.tile_pool(name="mps0", bufs=2, space="PSUM") as mps0_pool,
                tc.tile_pool(name="mps1", bufs=2, space="PSUM") as mps1_pool,
                tc.tile_pool(name="mtmp", bufs=2) as mtmp_pool,
                tc.tile_pool(name="xwt", bufs=3) as xwt_pool,
                tc.tile_pool(name="mdram", bufs=2, space="DRAM") as mdram_pool,
            ):
                xw_re = xw_dram.rearrange("(m p) t -> p m t", p=128)
                ag_re = None
                for t in range(s_steps):
                    xw_t = xwt_pool.tile([128, MT, B], DT.float32, tag="xwt")
                    nc.sync.dma_start(xw_t[:], xw_re[:, :, t * B : (t + 1) * B])

                    ag_in = mdram_pool.tile([2 * 128, B], DT.float16, tag="agin")
                    ps_pools = (mps0_pool, mps1_pool)
                    h_out16 = []
                    for s_ in range(2):
                        ps = ps_pools[s_].tile([128, 4, B], DT.float32, tag="ps")
                        for ml in range(4):  # g, i, f, o
                            m = s_ * 4 + ml
                            for k in range(KT_H):
                                nc.tensor.matmul(
                                    ps[:, ml, :],
                                    whh_sb[:, k, m, :],
                                    h_sb[:, k, :],
                                    start=(k == 0),
                                    stop=(k == KT_H - 1),
                                )
                        gpre = mtmp_pool.tile([128, 4, B], DT.float32, tag=f"gpre{s_}")
                        nc.vector.tensor_add(
                            gpre[:], ps[:], xw_t[:, s_ * 4 : s_ * 4 + 4, :]
                        )
                        gact = mtmp_pool.tile([128, 4, B], DT.float32, tag=f"gact{s_}")
                        nc.scalar.activation(
                            gact[:, 0, :], gpre[:, 0, :], AF.Tanh
                        )
                        nc.scalar.activation(
                            gact[:, 1:3, :], gpre[:, 1:3, :], AF.Sigmoid
                        )
                        nc.scalar.activation(
                            gact[:, 3, :], gpre[:, 3, :], AF.Sigmoid
                        )
                        g_v = gact[:, 0, :]
                        i_v = gact[:, 1, :]
                        f_v = gact[:, 2, :]
                        o_v = gact[:, 3, :]
                        t1 = mtmp_pool.tile([128, B], DT.float32, tag=f"t1{s_}")
                        t2 = mtmp_pool.tile([128, B], DT.float32, tag=f"t2{s_}")
                        nc.vector.tensor_mul(t1[:], i_v, g_v)
                        nc.vector.tensor_mul(t2[:], f_v, c_sb[:, s_, :])
                        nc.vector.tensor_add(c_sb[:, s_, :], t1[:], t2[:])
                        th = mtmp_pool.tile([128, B], DT.float32, tag=f"th{s_}")
                        nc.scalar.activation(th[:], c_sb[:, s_, :], AF.Tanh)
                        out_h = mtmp_pool.tile([128, B], DT.float32, tag=f"oh{s_}")
                        nc.vector.tensor_mul(out_h[:], o_v, th[:])
                        nc.sync.dma_start(
                            out[t][s_ * 128 : (s_ + 1) * 128, :], out_h[:]
                        )
                        h16 = mtmp_pool.tile([128, B], DT.float16, tag=f"h16{s_}")
                        if t < KL:
                            nc.vector.tensor_scalar_mul(
                                h16[:], out_h[:], mult_sb[:, t, s_ : s_ + 1]
                            )
                            nc.vector.tensor_scalar_mul(
                                c_sb[:, s_, :],
                                c_sb[:, s_, :],
                                mult_sb[:, t, s_ : s_ + 1],
                            )
                        else:
                            nc.vector.tensor_copy(h16[:], out_h[:])
                        nc.sync.dma_start(
                            ag_in[s_ * 128 : (s_ + 1) * 128, :], h16[:]
                        )
                        h_out16.append(h16)

                    ag_out = mdram_pool.tile(
                        [H, B], DT.float16, tag="agout", addr_space="Shared"
                    )
                    nc.gpsimd.collective_compute(
                        "AllGather",
                        ALU.bypass,
                        replica_groups=rg,
                        ins=[ag_in.opt()],
                        outs=[ag_out.opt()],
                    )
                    ag_re = ag_out.rearrange("(k p) b -> p k b", p=128)
                    for g in range(4):
                        nc.sync.dma_start(
                            h_sb[:, 4 * g : 4 * (g + 1), :],
                            ag_re[:, 4 * g : 4 * (g + 1), :],
                        )

    nc.compile()
    return nc


def _prepare_inputs(x, key_seq, weight_ih, weight_hh, bias_ih, bias_hh, s_steps):
    x = np.ascontiguousarray(np.asarray(x, dtype=np.float32)[:, :s_steps, :])
    key_seq = np.asarray(key_seq, dtype=np.float32)
    weight_ih = np.asarray(weight_ih, dtype=np.float32)
    weight_hh = np.asarray(weight_hh, dtype=np.float32)
    b = (np.asarray(bias_ih, dtype=np.float32) + np.asarray(bias_hh, dtype=np.float32))

    # tokens ordered (s, b): column s*B + b
    xt = np.ascontiguousarray(x.transpose(2, 1, 0).reshape(I, s_steps * B))
    kt = np.ascontiguousarray(key_seq.transpose(2, 1, 0).reshape(I, KL * KB))

    in_maps = []
    for j in range(NCORES):
        rows = _rows_for_core(j)
        in_maps.append(
            {
                "xt": xt,
                "kt": kt,
                "wih": np.ascontiguousarray(weight_ih[rows].T),
                "whh": np.ascontiguousarray(weight_hh[rows].T.astype(np.float16)),
                "bias": np.ascontiguousarray(b[rows]),
            }
        )
    return in_maps


_NC_CACHE = {}


def _run(x, key_seq, weight_ih, weight_hh, bias_ih, bias_hh, s_steps, trace=False):
    if s_steps not in _NC_CACHE:
        _NC_CACHE[s_steps] = _build_program(s_steps)
    nc = _NC_CACHE[s_steps]
    in_maps = _prepare_inputs(
        x, key_seq, weight_ih, weight_hh, bias_ih, bias_hh, s_steps
    )
    res = bass_utils.run_bass_kernel_spmd(
        nc, in_maps, core_ids=list(range(NCORES)), trace=trace
    )
    # out_j: [s, r, b] with global h row = j*HLOC + r
    pieces = [res.results[j]["out"].transpose(0, 2, 1) for j in range(NCORES)]
    full = np.concatenate(pieces, axis=2)  # (s_steps, B, H)
    return full, res


def kernel(x, key_seq, weight_ih, weight_hh, bias_ih, bias_hh):
    s_steps = int(os.environ.get("KEYED_LSTM_STEPS", S))
    trace = os.environ.get("KEYED_LSTM_TRACE", "0") == "1"
    full, _res = _run(
        x, key_seq, weight_ih, weight_hh, bias_ih, bias_hh, s_steps, trace=trace
    )
    return full
